# revision 9
# baseline (speedup 1.0000x reference)
"""Trainium2 Bass kernel for DecoupledTopKGate (moe_routing).

Computation (reference):
    sel = x @ w_sel.T                      # [8192, 64]
    fus = (x @ w_fus.T) * momentum[None]   # [8192, 64]
    tv, ti = top_k(sel, 2)                 # [8192, 2]
    counts = histogram(ti, 64)
    new_usage = 0.9*usage + 0.1*counts
    new_mw = softmax(1/(new_usage/(sum+eps)+eps))

Sharding: data-parallel over tokens (1024 tokens/core x 8 cores), gate
weights replicated, expert-count histogram all-reduced across cores on
device; EMA + softmax computed (redundantly) on every core.

Per-core plan:
  - host pre-packs wt = concat(w_sel.T, w_fus.T) -> [4096, 128] so one
    weight-stationary matmul computes both gates (M=128).
  - x arrives token-major; the contraction dim (d) must live on SBUF
    partitions, so x-tiles are transposed on-chip with PE transpose
    (fp32 has no DMA-transpose path), then fp32 matmuls with N=512
    accumulate 32 k-chunks into PSUM as scoresT [128(sel||fus), 512t].
  - momentum scaling is applied during the PSUM->SBUF copy via a
    per-partition scalar (rows 0:64 = 1.0, rows 64:128 = momentum).
  - scoresT is transposed back per 128-token block; top-2 via the DVE
    max/max_index (top-8) instructions; expert histogram via a one-hot
    mask contracted with a ones-vector on the PE into PSUM [1, 64].
  - tail: AllReduce(counts), EMA, normalize, reciprocal, softmax on
    [1, 64] rows, then DMA out.
"""

import numpy as np
from contextlib import ExitStack

import concourse.bass as bass
from concourse import bacc
import concourse.mybir as mybir
import concourse.tile as tile
from concourse.bass_utils import run_bass_kernel_spmd
from concourse.masks import make_identity

F32 = mybir.dt.float32
F32R = mybir.dt.float32r
I32 = mybir.dt.int32
U32 = mybir.dt.uint32

TOKENS = 8192
D = 4096
E = 64
TOP_K = 2
MOMENTUM = 0.9
EPS = 1e-8

NCORES = 8
TPC = TOKENS // NCORES            # 1024 tokens per core
BLK = 128                         # token block (partition dim)
NBLK = TPC // BLK                 # 8 blocks per core
ST = 512                          # super-tile (matmul moving dim N)
BLKS_PER_ST = ST // BLK           # 4
NST = TPC // ST                   # 2
KC = D // 128                     # 32 contraction chunks

# Tunables
USE_F32R = False     # float32r matmul: 4x PE rate but reduced precision (2xbf16)
DVE_COLS = 256       # PSUM->SBUF xT copy split: [0:DVE_COLS] on DVE, rest on ACT


def build_nc(use_f32r=USE_F32R, dve_cols=DVE_COLS):
    nc = bacc.Bacc(None, target_bir_lowering=False, debug=False,
                   num_devices=NCORES)

    x = nc.dram_tensor("x", [TPC, D], F32, kind="ExternalInput")
    wt = nc.dram_tensor("wt", [D, 128], F32, kind="ExternalInput")
    mw_col = nc.dram_tensor("mw_col", [128, 1], F32, kind="ExternalInput")
    usage_in = nc.dram_tensor("usage_in", [1, E], F32, kind="ExternalInput")

    sel_out = nc.dram_tensor("sel_out", [TPC, E], F32, kind="ExternalOutput")
    fus_out = nc.dram_tensor("fus_out", [TPC, E], F32, kind="ExternalOutput")
    tv_out = nc.dram_tensor("tv_out", [TPC, TOP_K], F32, kind="ExternalOutput")
    ti_out = nc.dram_tensor("ti_out", [TPC, TOP_K], I32, kind="ExternalOutput")
    usage_out = nc.dram_tensor("usage_out", [1, E], F32, kind="ExternalOutput")
    mw_out = nc.dram_tensor("mw_out", [1, E], F32, kind="ExternalOutput")

    with tile.TileContext(nc) as tc, ExitStack() as ctx:
        consts = ctx.enter_context(tc.tile_pool(name="consts", bufs=1))
        x_pool = ctx.enter_context(tc.tile_pool(name="x_pool", bufs=NBLK))
        xt_pool = ctx.enter_context(tc.tile_pool(name="xt_pool", bufs=3))
        sb_pool = ctx.enter_context(tc.tile_pool(name="sb_pool", bufs=3))
        sm_pool = ctx.enter_context(tc.tile_pool(name="sm_pool", bufs=2))
        xp_psum = ctx.enter_context(tc.tile_pool(name="xp_psum", bufs=2, space="PSUM"))
        sc_psum = ctx.enter_context(tc.tile_pool(name="sc_psum", bufs=2, space="PSUM"))
        tb_psum = ctx.enter_context(tc.tile_pool(name="tb_psum", bufs=2, space="PSUM"))
        ct_psum = ctx.enter_context(tc.tile_pool(name="ct_psum", bufs=1, space="PSUM"))
        dram = ctx.enter_context(tc.tile_pool(name="dram", bufs=1, space="DRAM"))

        # ---- constants ----
        identity = consts.tile([128, 128], F32)
        make_identity(nc, identity)
        iota_f = consts.tile([128, E], F32)
        nc.gpsimd.iota(iota_f, pattern=[[1, E]], base=0, channel_multiplier=0,
                       allow_small_or_imprecise_dtypes=True)
        ones_col = consts.tile([128, 1], F32)
        nc.vector.memset(ones_col, 1.0)

        # fused transposed weights: wt[d, e'] -> SBUF [128, KC, 128].
        # Bounce through a DVE copy so matmuls wait on the DVE semaphore
        # (shared with the xt copies) instead of DMA sems — walrus caps
        # the number of sync waits a Matmult can carry. For the f32r
        # variant the copy also performs the required f32r rounding.
        mm_dt = F32R if use_f32r else F32
        wt_raw = consts.tile([128, KC, 128], F32)
        nc.sync.dma_start(wt_raw, wt[:, :].rearrange("(k p) e -> p k e", p=128))
        wt_sb = consts.tile([128, KC, 128], mm_dt)
        nc.vector.tensor_copy(wt_sb, wt_raw)
        mw_sb = consts.tile([128, 1], F32)
        nc.sync.dma_start(mw_sb, mw_col[:, :])
        usage_sb = consts.tile([1, E], F32)
        nc.sync.dma_start(usage_sb, usage_in[:, :])

        # ---- prefetch all x blocks ----
        x_nat = []
        for b in range(NBLK):
            xb = x_pool.tile([128, D], F32, tag="x_nat", name=f"x_nat{b}")
            nc.sync.dma_start(xb, x[b * BLK:(b + 1) * BLK, :])
            x_nat.append(xb)

        counts_ps = ct_psum.tile([1, E], F32)

        for s in range(NST):
            blocks = list(range(s * BLKS_PER_ST, (s + 1) * BLKS_PER_ST))
            scores_ps = sc_psum.tile([128, ST], F32, tag="scores")
            for k in range(KC):
                xt_ps = xp_psum.tile([128, ST], F32, tag="xt_ps")
                for j, b in enumerate(blocks):
                    nc.tensor.transpose(
                        xt_ps[:, j * BLK:(j + 1) * BLK],
                        x_nat[b][:, k * 128:(k + 1) * 128],
                        identity,
                    )
                # Whole-tile copy by a single engine (so the consuming
                # matmul has one producer sem); alternate DVE/ACT per k
                # to split the PSUM->SBUF bandwidth between engines.
                xt_sb = xt_pool.tile([128, ST], mm_dt, tag="xt_sb")
                if k % 2 == 0:
                    nc.vector.tensor_copy(xt_sb, xt_ps)
                else:
                    nc.scalar.copy(xt_sb, xt_ps)
                nc.tensor.matmul(
                    scores_ps, wt_sb[:, k, :], xt_sb,
                    start=(k == 0), stop=(k == KC - 1),
                )

            # scoresT -> SBUF with momentum scaling on rows 64:128
            scT = sm_pool.tile([128, ST], F32, tag="scT")
            nc.vector.tensor_scalar_mul(scT, scores_ps, mw_sb)

            for j, b in enumerate(blocks):
                tb_ps = tb_psum.tile([128, 128], F32, tag="tb")
                nc.tensor.transpose(tb_ps, scT[:, j * BLK:(j + 1) * BLK], identity)
                tb_sb = sb_pool.tile([128, 128], F32, tag="tb_sb")
                nc.vector.tensor_copy(tb_sb, tb_ps)
                nc.sync.dma_start(sel_out[b * BLK:(b + 1) * BLK, :], tb_sb[:, 0:E])
                nc.sync.dma_start(fus_out[b * BLK:(b + 1) * BLK, :], tb_sb[:, E:128])

                max8 = sb_pool.tile([128, 8], F32, tag="max8")
                nc.vector.max(out=max8, in_=tb_sb[:, 0:E])
                idx8 = sb_pool.tile([128, 8], U32, tag="idx8")
                nc.vector.max_index(idx8, max8, tb_sb[:, 0:E])
                nc.sync.dma_start(tv_out[b * BLK:(b + 1) * BLK, :], max8[:, 0:TOP_K])
                nc.sync.dma_start(ti_out[b * BLK:(b + 1) * BLK, :],
                                  idx8[:, 0:TOP_K].bitcast(I32))

                idxf = sb_pool.tile([128, TOP_K], F32, tag="idxf")
                nc.vector.tensor_copy(idxf, idx8[:, 0:TOP_K])
                mask = sb_pool.tile([128, E], F32, tag="mask")
                nc.vector.tensor_scalar(mask, iota_f, idxf[:, 0:1], None,
                                        op0=mybir.AluOpType.is_equal)
                mask2 = sb_pool.tile([128, E], F32, tag="mask2")
                nc.vector.tensor_scalar(mask2, iota_f, idxf[:, 1:2], None,
                                        op0=mybir.AluOpType.is_equal)
                nc.vector.tensor_add(mask, mask, mask2)
                nc.tensor.matmul(
                    counts_ps, ones_col, mask,
                    start=(b == 0), stop=(b == NBLK - 1),
                    skip_group_check=True,
                )

        # ---- tail: all-reduce counts, EMA, softmax ----
        cc_in = dram.tile([1, E], F32)
        cc_out = dram.tile([1, E], F32, addr_space="Shared")
        counts_sb = sb_pool.tile([1, E], F32)
        nc.vector.tensor_copy(counts_sb, counts_ps)
        nc.sync.dma_start(cc_in, counts_sb)
        nc.gpsimd.collective_compute(
            "AllReduce", mybir.AluOpType.add,
            replica_groups=[list(range(NCORES))],
            ins=[cc_in.opt()], outs=[cc_out.opt()],
        )
        cr = sb_pool.tile([1, E], F32)
        nc.sync.dma_start(cr, cc_out)

        new_usage = sb_pool.tile([1, E], F32)
        # new_usage = usage*0.9 + counts*(1-0.9)
        tmp = sb_pool.tile([1, E], F32)
        nc.vector.tensor_scalar_mul(tmp, cr, 1.0 - MOMENTUM)
        nc.vector.tensor_scalar_mul(new_usage, usage_sb, MOMENTUM)
        nc.vector.tensor_add(new_usage, new_usage, tmp)
        nc.sync.dma_start(usage_out[:, :], new_usage)

        ssum = sb_pool.tile([1, 1], F32)
        nc.vector.reduce_sum(out=ssum, in_=new_usage, axis=mybir.AxisListType.X)
        nc.vector.tensor_scalar_add(ssum, ssum, EPS)
        rsum = sb_pool.tile([1, 1], F32)
        nc.vector.reciprocal(rsum, ssum)
        norm = sb_pool.tile([1, E], F32)
        nc.vector.tensor_scalar(norm, new_usage, rsum, EPS,
                                op0=mybir.AluOpType.mult,
                                op1=mybir.AluOpType.add)
        inv = sb_pool.tile([1, E], F32)
        nc.vector.reciprocal(inv, norm)
        mx = sb_pool.tile([1, 1], F32)
        nc.vector.reduce_max(out=mx, in_=inv, axis=mybir.AxisListType.X)
        z = sb_pool.tile([1, E], F32)
        nc.vector.tensor_scalar(z, inv, mx, None, op0=mybir.AluOpType.subtract)
        ez = sb_pool.tile([1, E], F32)
        nc.scalar.activation(ez, z, mybir.ActivationFunctionType.Exp)
        esum = sb_pool.tile([1, 1], F32)
        nc.vector.reduce_sum(out=esum, in_=ez, axis=mybir.AxisListType.X)
        resum = sb_pool.tile([1, 1], F32)
        nc.vector.reciprocal(resum, esum)
        mw_new = sb_pool.tile([1, E], F32)
        nc.vector.tensor_scalar_mul(mw_new, ez, resum)
        nc.sync.dma_start(mw_out[:, :], mw_new)

    nc.compile()
    return nc


_NC_CACHE = {}


def _get_nc(use_f32r=USE_F32R, dve_cols=DVE_COLS):
    key = (use_f32r, dve_cols)
    if key not in _NC_CACHE:
        _NC_CACHE[key] = build_nc(use_f32r, dve_cols)
    return _NC_CACHE[key]


def _make_in_maps(x, w_sel, w_fus, momentum_weights, expert_usage_count):
    x = np.ascontiguousarray(np.asarray(x, dtype=np.float32))
    w_sel = np.asarray(w_sel, dtype=np.float32)
    w_fus = np.asarray(w_fus, dtype=np.float32)
    mw = np.asarray(momentum_weights, dtype=np.float32).reshape(E)
    usage = np.asarray(expert_usage_count, dtype=np.float32).reshape(1, E)

    wt = np.ascontiguousarray(
        np.concatenate([w_sel.T, w_fus.T], axis=1), dtype=np.float32
    )  # [D, 128]
    mw_col = np.concatenate([np.ones(E, np.float32), mw]).reshape(128, 1)

    return [
        {
            "x": x[c * TPC:(c + 1) * TPC],
            "wt": wt,
            "mw_col": mw_col,
            "usage_in": usage,
        }
        for c in range(NCORES)
    ]


def _run(in_maps, use_f32r=USE_F32R, dve_cols=DVE_COLS, **kwargs):
    nc = _get_nc(use_f32r, dve_cols)
    return run_bass_kernel_spmd(nc, in_maps, core_ids=list(range(NCORES)), **kwargs)


def _assemble(results):
    sel = np.concatenate([r["sel_out"] for r in results], axis=0)
    fus = np.concatenate([r["fus_out"] for r in results], axis=0)
    tv = np.concatenate([r["tv_out"] for r in results], axis=0)
    ti = np.concatenate([r["ti_out"] for r in results], axis=0).astype(np.int32)
    mw_new = results[0]["mw_out"].reshape(E)
    usage_new = results[0]["usage_out"].reshape(E)
    return sel, fus, tv, ti, mw_new, usage_new


def kernel(x, w_sel, w_fus, momentum_weights, expert_usage_count):
    in_maps = _make_in_maps(x, w_sel, w_fus, momentum_weights, expert_usage_count)
    res = _run(in_maps)
    return _assemble(res.results)


# revision 10
# speedup vs baseline: 1.1240x; 1.1240x over previous
"""Trainium2 Bass kernel for DecoupledTopKGate (moe_routing).

Computation (reference):
    sel = x @ w_sel.T                      # [8192, 64]
    fus = (x @ w_fus.T) * momentum[None]   # [8192, 64]
    tv, ti = top_k(sel, 2)                 # [8192, 2]
    counts = histogram(ti, 64)
    new_usage = 0.9*usage + 0.1*counts
    new_mw = softmax(1/(new_usage/(sum+eps)+eps))

Sharding: data-parallel over tokens (1024 tokens/core x 8 cores); the
per-core x shard is marshaled feature-major (d on SBUF partitions) on
the host, since the TensorE contraction dim must live on partitions.
Gate weights are replicated (host-packed as concat(w_sel.T, w_fus.T) so
one weight-stationary matmul computes both gates, M=128). The expert
count histogram is AllReduce'd across cores on device; the EMA +
softmax tail is computed (redundantly) on every core.

Per-core dataflow:
  - 8 DMAs stream xT [128, 32k, 1024t] into SBUF (4KB runs/partition).
  - fp32 matmuls, N=512 moving: for each 512-token super-tile,
    accumulate 32 k-chunks into PSUM as scoresT [128(sel||fus), 512].
  - momentum scaling happens in the PSUM->SBUF copy via a per-partition
    scalar (rows 0:64 = 1.0, rows 64:128 = momentum).
  - scoresT is transposed back per 128-token block (PE transpose);
    top-2 via the DVE max/max_index (top-8) instructions; histogram
    via a one-hot mask contracted with a ones column on the PE into a
    PSUM [1, 64] accumulator.
  - tail: AllReduce(counts) -> EMA -> normalize -> reciprocal ->
    softmax on [1, 64] rows, DMA out.
"""

import numpy as np
from contextlib import ExitStack

import concourse.bass as bass
from concourse import bacc
import concourse.mybir as mybir
import concourse.tile as tile
from concourse.bass_utils import run_bass_kernel_spmd
from concourse.masks import make_identity

F32 = mybir.dt.float32
I32 = mybir.dt.int32
U32 = mybir.dt.uint32

TOKENS = 8192
D = 4096
E = 64
TOP_K = 2
MOMENTUM = 0.9
EPS = 1e-8

NCORES = 8
TPC = TOKENS // NCORES            # 1024 tokens per core
BLK = 128                         # token block (partition dim)
NBLK = TPC // BLK                 # 8 blocks per core
ST = 512                          # super-tile (matmul moving dim N)
BLKS_PER_ST = ST // BLK           # 4
NST = TPC // ST                   # 2
KC = D // 128                     # 32 contraction chunks
KG = 8                            # xT arrives in KG DMA pieces of KC/KG chunks


def build_nc():
    nc = bacc.Bacc(None, target_bir_lowering=False, debug=False,
                   num_devices=NCORES)

    xt = nc.dram_tensor("xt", [D, TPC], F32, kind="ExternalInput")
    wt = nc.dram_tensor("wt", [D, 128], F32, kind="ExternalInput")
    mw_col = nc.dram_tensor("mw_col", [128, 1], F32, kind="ExternalInput")
    usage_in = nc.dram_tensor("usage_in", [1, E], F32, kind="ExternalInput")

    sel_out = nc.dram_tensor("sel_out", [TPC, E], F32, kind="ExternalOutput")
    fus_out = nc.dram_tensor("fus_out", [TPC, E], F32, kind="ExternalOutput")
    tv_out = nc.dram_tensor("tv_out", [TPC, TOP_K], F32, kind="ExternalOutput")
    ti_out = nc.dram_tensor("ti_out", [TPC, TOP_K], I32, kind="ExternalOutput")
    usage_out = nc.dram_tensor("usage_out", [1, E], F32, kind="ExternalOutput")
    mw_out = nc.dram_tensor("mw_out", [1, E], F32, kind="ExternalOutput")

    with tile.TileContext(nc) as tc, ExitStack() as ctx:
        consts = ctx.enter_context(tc.tile_pool(name="consts", bufs=1))
        sb_pool = ctx.enter_context(tc.tile_pool(name="sb_pool", bufs=3))
        sm_pool = ctx.enter_context(tc.tile_pool(name="sm_pool", bufs=2))
        sc_psum = ctx.enter_context(tc.tile_pool(name="sc_psum", bufs=2, space="PSUM"))
        tb_psum = ctx.enter_context(tc.tile_pool(name="tb_psum", bufs=2, space="PSUM"))
        ct_psum = ctx.enter_context(tc.tile_pool(name="ct_psum", bufs=1, space="PSUM"))
        dram = ctx.enter_context(tc.tile_pool(name="dram", bufs=1, space="DRAM"))

        # ---- x shard, feature-major: [128, KC, TPC], streamed in KG pieces
        # (4KB contiguous per partition-row per chunk) so matmuls start
        # after the first piece lands.
        xt_sb = consts.tile([128, KC, TPC], F32)
        xt_ap = xt[:, :].rearrange("(k p) t -> p k t", p=128)
        kper = KC // KG
        for g in range(KG):
            nc.sync.dma_start(xt_sb[:, g * kper:(g + 1) * kper, :],
                              xt_ap[:, g * kper:(g + 1) * kper, :])

        # fused transposed weights: wt[d, e'] -> SBUF [128, KC, 128].
        # Bounced through a DVE copy so the consuming matmuls wait on the
        # DVE semaphore instead of extra DMA lanes (PE instructions have
        # very limited sync-wait slots; Bacc legalizes, but fewer event
        # semaphores on the PE queue is still cheaper).
        wt_raw = consts.tile([128, KC, 128], F32)
        nc.sync.dma_start(wt_raw, wt[:, :].rearrange("(k p) e -> p k e", p=128))
        wt_sb = consts.tile([128, KC, 128], F32)
        nc.vector.tensor_copy(wt_sb, wt_raw)

        # ---- small constants (SWDGE path; keeps the HWDGE queue free
        # for the big streaming loads — a [128,1] column load costs ~7us
        # in 4-byte descriptors and must not block the x stream) ----
        identity = consts.tile([128, 128], F32)
        make_identity(nc, identity)
        iota_f = consts.tile([128, E], F32)
        nc.gpsimd.iota(iota_f, pattern=[[1, E]], base=0, channel_multiplier=0,
                       allow_small_or_imprecise_dtypes=True)
        ones_col = consts.tile([128, 1], F32)
        nc.vector.memset(ones_col, 1.0)
        mw_sb = consts.tile([128, 1], F32)
        nc.gpsimd.dma_start(mw_sb, mw_col[:, :])
        usage_sb = consts.tile([1, E], F32)
        nc.gpsimd.dma_start(usage_sb, usage_in[:, :])

        counts_ps = ct_psum.tile([1, E], F32)

        for s in range(NST):
            scores_ps = sc_psum.tile([128, ST], F32, tag="scores")
            for k in range(KC):
                nc.tensor.matmul(
                    scores_ps,
                    wt_sb[:, k, :],
                    xt_sb[:, k, s * ST:(s + 1) * ST],
                    start=(k == 0), stop=(k == KC - 1),
                )

            # scoresT -> SBUF with momentum scaling on rows 64:128
            scT = sm_pool.tile([128, ST], F32, tag="scT")
            nc.vector.tensor_scalar_mul(scT, scores_ps, mw_sb)

            for j in range(BLKS_PER_ST):
                b = s * BLKS_PER_ST + j
                tb_ps = tb_psum.tile([128, 128], F32, tag="tb")
                nc.tensor.transpose(tb_ps, scT[:, j * BLK:(j + 1) * BLK], identity)
                tb_sb = sb_pool.tile([128, 128], F32, tag="tb_sb")
                nc.vector.tensor_copy(tb_sb, tb_ps)
                nc.sync.dma_start(sel_out[b * BLK:(b + 1) * BLK, :], tb_sb[:, 0:E])
                nc.sync.dma_start(fus_out[b * BLK:(b + 1) * BLK, :], tb_sb[:, E:128])

                max8 = sb_pool.tile([128, 8], F32, tag="max8")
                nc.vector.max(out=max8, in_=tb_sb[:, 0:E])
                idx8 = sb_pool.tile([128, 8], U32, tag="idx8")
                nc.vector.max_index(idx8, max8, tb_sb[:, 0:E])
                nc.sync.dma_start(tv_out[b * BLK:(b + 1) * BLK, :], max8[:, 0:TOP_K])
                nc.sync.dma_start(ti_out[b * BLK:(b + 1) * BLK, :],
                                  idx8[:, 0:TOP_K].bitcast(I32))

                idxf = sb_pool.tile([128, TOP_K], F32, tag="idxf")
                nc.vector.tensor_copy(idxf, idx8[:, 0:TOP_K])
                mask = sb_pool.tile([128, E], F32, tag="mask")
                nc.vector.tensor_scalar(mask, iota_f, idxf[:, 0:1], None,
                                        op0=mybir.AluOpType.is_equal)
                mask2 = sb_pool.tile([128, E], F32, tag="mask2")
                nc.vector.tensor_scalar(mask2, iota_f, idxf[:, 1:2], None,
                                        op0=mybir.AluOpType.is_equal)
                nc.vector.tensor_add(mask, mask, mask2)
                nc.tensor.matmul(
                    counts_ps, ones_col, mask,
                    start=(b == 0), stop=(b == NBLK - 1),
                    skip_group_check=True,
                )

        # ---- tail: all-reduce counts, EMA, softmax ----
        cc_in = dram.tile([1, E], F32)
        cc_out = dram.tile([1, E], F32, addr_space="Shared")
        counts_sb = sb_pool.tile([1, E], F32)
        nc.vector.tensor_copy(counts_sb, counts_ps)
        nc.sync.dma_start(cc_in, counts_sb)
        nc.gpsimd.collective_compute(
            "AllReduce", mybir.AluOpType.add,
            replica_groups=[list(range(NCORES))],
            ins=[cc_in.opt()], outs=[cc_out.opt()],
        )
        cr = sb_pool.tile([1, E], F32)
        nc.sync.dma_start(cr, cc_out)

        new_usage = sb_pool.tile([1, E], F32)
        # new_usage = usage*0.9 + counts*(1-0.9)
        tmp = sb_pool.tile([1, E], F32)
        nc.vector.tensor_scalar_mul(tmp, cr, 1.0 - MOMENTUM)
        nc.vector.tensor_scalar_mul(new_usage, usage_sb, MOMENTUM)
        nc.vector.tensor_add(new_usage, new_usage, tmp)
        nc.sync.dma_start(usage_out[:, :], new_usage)

        ssum = sb_pool.tile([1, 1], F32)
        nc.vector.reduce_sum(out=ssum, in_=new_usage, axis=mybir.AxisListType.X)
        nc.vector.tensor_scalar_add(ssum, ssum, EPS)
        rsum = sb_pool.tile([1, 1], F32)
        nc.vector.reciprocal(rsum, ssum)
        norm = sb_pool.tile([1, E], F32)
        nc.vector.tensor_scalar(norm, new_usage, rsum, EPS,
                                op0=mybir.AluOpType.mult,
                                op1=mybir.AluOpType.add)
        inv = sb_pool.tile([1, E], F32)
        nc.vector.reciprocal(inv, norm)
        mx = sb_pool.tile([1, 1], F32)
        nc.vector.reduce_max(out=mx, in_=inv, axis=mybir.AxisListType.X)
        z = sb_pool.tile([1, E], F32)
        nc.vector.tensor_scalar(z, inv, mx, None, op0=mybir.AluOpType.subtract)
        ez = sb_pool.tile([1, E], F32)
        nc.scalar.activation(ez, z, mybir.ActivationFunctionType.Exp)
        esum = sb_pool.tile([1, 1], F32)
        nc.vector.reduce_sum(out=esum, in_=ez, axis=mybir.AxisListType.X)
        resum = sb_pool.tile([1, 1], F32)
        nc.vector.reciprocal(resum, esum)
        mw_new = sb_pool.tile([1, E], F32)
        nc.vector.tensor_scalar_mul(mw_new, ez, resum)
        nc.sync.dma_start(mw_out[:, :], mw_new)

    nc.compile()
    return nc


_NC_CACHE = {}


def _get_nc():
    if "nc" not in _NC_CACHE:
        _NC_CACHE["nc"] = build_nc()
    return _NC_CACHE["nc"]


def _make_in_maps(x, w_sel, w_fus, momentum_weights, expert_usage_count):
    x = np.asarray(x, dtype=np.float32)
    w_sel = np.asarray(w_sel, dtype=np.float32)
    w_fus = np.asarray(w_fus, dtype=np.float32)
    mw = np.asarray(momentum_weights, dtype=np.float32).reshape(E)
    usage = np.asarray(expert_usage_count, dtype=np.float32).reshape(1, E)

    # feature-major marshaling of the token shards
    xT = x.T  # [D, TOKENS] view
    wt = np.ascontiguousarray(
        np.concatenate([w_sel.T, w_fus.T], axis=1), dtype=np.float32
    )  # [D, 128]
    mw_col = np.concatenate([np.ones(E, np.float32), mw]).reshape(128, 1)

    return [
        {
            "xt": np.ascontiguousarray(xT[:, c * TPC:(c + 1) * TPC]),
            "wt": wt,
            "mw_col": mw_col,
            "usage_in": usage,
        }
        for c in range(NCORES)
    ]


def _run(in_maps, **kwargs):
    nc = _get_nc()
    return run_bass_kernel_spmd(nc, in_maps, core_ids=list(range(NCORES)), **kwargs)


def _assemble(results):
    sel = np.concatenate([r["sel_out"] for r in results], axis=0)
    fus = np.concatenate([r["fus_out"] for r in results], axis=0)
    tv = np.concatenate([r["tv_out"] for r in results], axis=0)
    ti = np.concatenate([r["ti_out"] for r in results], axis=0).astype(np.int32)
    mw_new = results[0]["mw_out"].reshape(E)
    usage_new = results[0]["usage_out"].reshape(E)
    return sel, fus, tv, ti, mw_new, usage_new


def kernel(x, w_sel, w_fus, momentum_weights, expert_usage_count):
    in_maps = _make_in_maps(x, w_sel, w_fus, momentum_weights, expert_usage_count)
    res = _run(in_maps)
    return _assemble(res.results)


# revision 12
# speedup vs baseline: 1.4817x; 1.3182x over previous
"""Trainium2 Bass kernel for DecoupledTopKGate (moe_routing).

Computation (reference):
    sel = x @ w_sel.T                      # [8192, 64]
    fus = (x @ w_fus.T) * momentum[None]   # [8192, 64]
    tv, ti = top_k(sel, 2)                 # [8192, 2]
    counts = histogram(ti, 64)
    new_usage = 0.9*usage + 0.1*counts
    new_mw = softmax(1/(new_usage/(sum+eps)+eps))

Sharding: data-parallel over tokens (1024 tokens/core x 8 cores); the
per-core x shard is marshaled feature-major (d on SBUF partitions) on
the host, since the TensorE contraction dim must live on partitions.
Gate weights are replicated (host-packed as concat(w_sel.T, w_fus.T) so
one weight-stationary matmul computes both gates, M=128). The expert
count histogram is AllReduce'd across cores on device; the EMA +
softmax tail is computed (redundantly) on every core.

Per-core dataflow:
  - 8 DMAs stream xT [128, 32k, 1024t] into SBUF (4KB runs/partition).
  - fp32 matmuls, N=512 moving: for each 512-token super-tile,
    accumulate 32 k-chunks into PSUM as scoresT [128(sel||fus), 512].
  - momentum scaling happens in the PSUM->SBUF copy via a per-partition
    scalar (rows 0:64 = 1.0, rows 64:128 = momentum).
  - scoresT is transposed back per 128-token block (PE transpose);
    top-2 via the DVE max/max_index (top-8) instructions; histogram
    via a one-hot mask contracted with a ones column on the PE into a
    PSUM [1, 64] accumulator.
  - tail: AllReduce(counts) -> EMA -> normalize -> reciprocal ->
    softmax on [1, 64] rows, DMA out.
"""

import numpy as np
from contextlib import ExitStack

import concourse.bass as bass
from concourse import bacc
import concourse.mybir as mybir
import concourse.tile as tile
from concourse.bass_utils import run_bass_kernel_spmd
from concourse.masks import make_identity

F32 = mybir.dt.float32
I32 = mybir.dt.int32
U32 = mybir.dt.uint32

TOKENS = 8192
D = 4096
E = 64
TOP_K = 2
MOMENTUM = 0.9
EPS = 1e-8

NCORES = 8
TPC = TOKENS // NCORES            # 1024 tokens per core
BLK = 128                         # token block (partition dim)
NBLK = TPC // BLK                 # 8 blocks per core
ST = 512                          # super-tile (matmul moving dim N)
BLKS_PER_ST = ST // BLK           # 4
NST = TPC // ST                   # 2
KC = D // 128                     # 32 contraction chunks
KG = 8                            # xT arrives in KG DMA pieces of KC/KG chunks


def build_nc():
    nc = bacc.Bacc(None, target_bir_lowering=False, debug=False,
                   num_devices=NCORES)

    xt = nc.dram_tensor("xt", [D, TPC], F32, kind="ExternalInput")
    wt = nc.dram_tensor("wt", [D, 128], F32, kind="ExternalInput")
    mw_col = nc.dram_tensor("mw_col", [128, 1], F32, kind="ExternalInput")
    usage_in = nc.dram_tensor("usage_in", [1, E], F32, kind="ExternalInput")

    sel_out = nc.dram_tensor("sel_out", [TPC, E], F32, kind="ExternalOutput")
    fus_out = nc.dram_tensor("fus_out", [TPC, E], F32, kind="ExternalOutput")
    tv_out = nc.dram_tensor("tv_out", [TPC, TOP_K], F32, kind="ExternalOutput")
    ti_out = nc.dram_tensor("ti_out", [TPC, TOP_K], I32, kind="ExternalOutput")
    usage_out = nc.dram_tensor("usage_out", [1, E], F32, kind="ExternalOutput")
    mw_out = nc.dram_tensor("mw_out", [1, E], F32, kind="ExternalOutput")

    with tile.TileContext(nc) as tc, ExitStack() as ctx:
        consts = ctx.enter_context(tc.tile_pool(name="consts", bufs=1))
        sb_pool = ctx.enter_context(tc.tile_pool(name="sb_pool", bufs=3))
        sm_pool = ctx.enter_context(tc.tile_pool(name="sm_pool", bufs=2))
        sc_psum = ctx.enter_context(tc.tile_pool(name="sc_psum", bufs=2, space="PSUM"))
        tb_psum = ctx.enter_context(tc.tile_pool(name="tb_psum", bufs=2, space="PSUM"))
        ct_psum = ctx.enter_context(tc.tile_pool(name="ct_psum", bufs=1, space="PSUM"))
        dram = ctx.enter_context(tc.tile_pool(name="dram", bufs=1, space="DRAM"))

        # fused transposed weights FIRST (2MB, gates the first matmul —
        # must not queue behind the 16MB x stream): wt[d, e'] -> SBUF
        # [128, KC, 128]. Bounced through a DVE copy so the consuming
        # matmuls wait on the DVE semaphore instead of extra DMA lanes
        # (PE instructions have very limited sync-wait slots; Bacc
        # legalizes, but fewer event semaphores on the PE queue is
        # still cheaper).
        wt_raw = consts.tile([128, KC, 128], F32)
        nc.sync.dma_start(wt_raw, wt[:, :].rearrange("(k p) e -> p k e", p=128))
        wt_sb = consts.tile([128, KC, 128], F32)
        nc.vector.tensor_copy(wt_sb, wt_raw)

        # ---- x shard, feature-major: [128, KC, TPC], streamed in KG pieces
        # (4KB contiguous per partition-row per chunk) so matmuls start
        # after the first piece lands.
        xt_sb = consts.tile([128, KC, TPC], F32)
        xt_ap = xt[:, :].rearrange("(k p) t -> p k t", p=128)
        kper = KC // KG
        for g in range(KG):
            nc.sync.dma_start(xt_sb[:, g * kper:(g + 1) * kper, :],
                              xt_ap[:, g * kper:(g + 1) * kper, :])

        # ---- small constants (SWDGE path; keeps the HWDGE queue free
        # for the big streaming loads — a [128,1] column load costs ~7us
        # in 4-byte descriptors and must not block the x stream) ----
        identity = consts.tile([128, 128], F32)
        make_identity(nc, identity)
        iota_f = consts.tile([128, E], F32)
        nc.gpsimd.iota(iota_f, pattern=[[1, E]], base=0, channel_multiplier=0,
                       allow_small_or_imprecise_dtypes=True)
        ones_col = consts.tile([128, 1], F32)
        nc.vector.memset(ones_col, 1.0)
        mw_sb = consts.tile([128, 1], F32)
        nc.gpsimd.dma_start(mw_sb, mw_col[:, :])
        usage_sb = consts.tile([1, E], F32)
        nc.gpsimd.dma_start(usage_sb, usage_in[:, :])

        counts_ps = ct_psum.tile([1, E], F32)

        # PE warm-up: the HAM clock gate needs ~3.4us of sustained PE
        # activity to release the 1.2->2.4 GHz throttle. Burn identity
        # matmuls into a scratch bank while the x stream fills SBUF so
        # the real matmuls start warm.
        warm_psum = ctx.enter_context(tc.tile_pool(name="warm", bufs=1, space="PSUM"))
        warm_ps = warm_psum.tile([128, 128], F32)
        for _ in range(72):
            nc.tensor.matmul(warm_ps, identity, identity, start=True, stop=True,
                             skip_group_check=True)

        for s in range(NST):
            scores_ps = sc_psum.tile([128, ST], F32, tag="scores")
            for k in range(KC):
                nc.tensor.matmul(
                    scores_ps,
                    wt_sb[:, k, :],
                    xt_sb[:, k, s * ST:(s + 1) * ST],
                    start=(k == 0), stop=(k == KC - 1),
                )

            # scoresT -> SBUF with momentum scaling on rows 64:128
            scT = sm_pool.tile([128, ST], F32, tag="scT")
            nc.vector.tensor_scalar_mul(scT, scores_ps, mw_sb)

            for j in range(BLKS_PER_ST):
                b = s * BLKS_PER_ST + j
                tb_ps = tb_psum.tile([128, 128], F32, tag="tb")
                nc.tensor.transpose(tb_ps, scT[:, j * BLK:(j + 1) * BLK], identity)
                tb_sb = sb_pool.tile([128, 128], F32, tag="tb_sb")
                nc.vector.tensor_copy(tb_sb, tb_ps)
                nc.sync.dma_start(sel_out[b * BLK:(b + 1) * BLK, :], tb_sb[:, 0:E])
                nc.sync.dma_start(fus_out[b * BLK:(b + 1) * BLK, :], tb_sb[:, E:128])

                max8 = sb_pool.tile([128, 8], F32, tag="max8")
                nc.vector.max(out=max8, in_=tb_sb[:, 0:E])
                idx8 = sb_pool.tile([128, 8], U32, tag="idx8")
                nc.vector.max_index(idx8, max8, tb_sb[:, 0:E])
                nc.sync.dma_start(tv_out[b * BLK:(b + 1) * BLK, :], max8[:, 0:TOP_K])
                nc.sync.dma_start(ti_out[b * BLK:(b + 1) * BLK, :],
                                  idx8[:, 0:TOP_K].bitcast(I32))

                idxf = sb_pool.tile([128, TOP_K], F32, tag="idxf")
                nc.vector.tensor_copy(idxf, idx8[:, 0:TOP_K])
                mask = sb_pool.tile([128, E], F32, tag="mask")
                nc.vector.tensor_scalar(mask, iota_f, idxf[:, 0:1], None,
                                        op0=mybir.AluOpType.is_equal)
                mask2 = sb_pool.tile([128, E], F32, tag="mask2")
                nc.vector.tensor_scalar(mask2, iota_f, idxf[:, 1:2], None,
                                        op0=mybir.AluOpType.is_equal)
                nc.vector.tensor_add(mask, mask, mask2)
                nc.tensor.matmul(
                    counts_ps, ones_col, mask,
                    start=(b == 0), stop=(b == NBLK - 1),
                    skip_group_check=True,
                )

        # ---- tail: all-reduce counts, EMA, softmax ----
        cc_in = dram.tile([1, E], F32)
        cc_out = dram.tile([1, E], F32, addr_space="Shared")
        counts_sb = sb_pool.tile([1, E], F32)
        nc.vector.tensor_copy(counts_sb, counts_ps)
        nc.sync.dma_start(cc_in, counts_sb)
        nc.gpsimd.collective_compute(
            "AllReduce", mybir.AluOpType.add,
            replica_groups=[list(range(NCORES))],
            ins=[cc_in.opt()], outs=[cc_out.opt()],
        )
        cr = sb_pool.tile([1, E], F32)
        nc.sync.dma_start(cr, cc_out)

        new_usage = sb_pool.tile([1, E], F32)
        # new_usage = usage*0.9 + counts*(1-0.9)
        tmp = sb_pool.tile([1, E], F32)
        nc.vector.tensor_scalar_mul(tmp, cr, 1.0 - MOMENTUM)
        nc.vector.tensor_scalar_mul(new_usage, usage_sb, MOMENTUM)
        nc.vector.tensor_add(new_usage, new_usage, tmp)
        nc.sync.dma_start(usage_out[:, :], new_usage)

        ssum = sb_pool.tile([1, 1], F32)
        nc.vector.reduce_sum(out=ssum, in_=new_usage, axis=mybir.AxisListType.X)
        nc.vector.tensor_scalar_add(ssum, ssum, EPS)
        rsum = sb_pool.tile([1, 1], F32)
        nc.vector.reciprocal(rsum, ssum)
        norm = sb_pool.tile([1, E], F32)
        nc.vector.tensor_scalar(norm, new_usage, rsum, EPS,
                                op0=mybir.AluOpType.mult,
                                op1=mybir.AluOpType.add)
        inv = sb_pool.tile([1, E], F32)
        nc.vector.reciprocal(inv, norm)
        mx = sb_pool.tile([1, 1], F32)
        nc.vector.reduce_max(out=mx, in_=inv, axis=mybir.AxisListType.X)
        z = sb_pool.tile([1, E], F32)
        nc.vector.tensor_scalar(z, inv, mx, None, op0=mybir.AluOpType.subtract)
        ez = sb_pool.tile([1, E], F32)
        nc.scalar.activation(ez, z, mybir.ActivationFunctionType.Exp)
        esum = sb_pool.tile([1, 1], F32)
        nc.vector.reduce_sum(out=esum, in_=ez, axis=mybir.AxisListType.X)
        resum = sb_pool.tile([1, 1], F32)
        nc.vector.reciprocal(resum, esum)
        mw_new = sb_pool.tile([1, E], F32)
        nc.vector.tensor_scalar_mul(mw_new, ez, resum)
        nc.sync.dma_start(mw_out[:, :], mw_new)

    nc.compile()
    return nc


_NC_CACHE = {}


def _get_nc():
    if "nc" not in _NC_CACHE:
        _NC_CACHE["nc"] = build_nc()
    return _NC_CACHE["nc"]


def _make_in_maps(x, w_sel, w_fus, momentum_weights, expert_usage_count):
    x = np.asarray(x, dtype=np.float32)
    w_sel = np.asarray(w_sel, dtype=np.float32)
    w_fus = np.asarray(w_fus, dtype=np.float32)
    mw = np.asarray(momentum_weights, dtype=np.float32).reshape(E)
    usage = np.asarray(expert_usage_count, dtype=np.float32).reshape(1, E)

    # feature-major marshaling of the token shards
    xT = x.T  # [D, TOKENS] view
    wt = np.ascontiguousarray(
        np.concatenate([w_sel.T, w_fus.T], axis=1), dtype=np.float32
    )  # [D, 128]
    mw_col = np.concatenate([np.ones(E, np.float32), mw]).reshape(128, 1)

    return [
        {
            "xt": np.ascontiguousarray(xT[:, c * TPC:(c + 1) * TPC]),
            "wt": wt,
            "mw_col": mw_col,
            "usage_in": usage,
        }
        for c in range(NCORES)
    ]


def _run(in_maps, **kwargs):
    nc = _get_nc()
    return run_bass_kernel_spmd(nc, in_maps, core_ids=list(range(NCORES)), **kwargs)


def _assemble(results):
    sel = np.concatenate([r["sel_out"] for r in results], axis=0)
    fus = np.concatenate([r["fus_out"] for r in results], axis=0)
    tv = np.concatenate([r["tv_out"] for r in results], axis=0)
    ti = np.concatenate([r["ti_out"] for r in results], axis=0).astype(np.int32)
    mw_new = results[0]["mw_out"].reshape(E)
    usage_new = results[0]["usage_out"].reshape(E)
    return sel, fus, tv, ti, mw_new, usage_new


def kernel(x, w_sel, w_fus, momentum_weights, expert_usage_count):
    in_maps = _make_in_maps(x, w_sel, w_fus, momentum_weights, expert_usage_count)
    res = _run(in_maps)
    return _assemble(res.results)
